# revision 18
# baseline (speedup 1.0000x reference)
"""Trainium2 Bass kernel for the NM multivariate-Gaussian-mixture benchmark.

Computes logsumexp_n log(sum_p exp(logprob[p, n])) for N=131072 samples,
K=8 clusters (P=64 ordered cluster pairs folded to 36 unordered pairs by
symmetry), D=32 dims.

Math (host precompute, float64):
  A_k   = L_k L_k^T + I,  Lc_k = chol(A_k),  B_k = inv(Lc_k)  (lower tri)
  For pair p=(i,j), i<=j:
    W_p  = B_i + B_j          (= inv(sigma_p), lower tri)
    mu_p = solve(W_p, B_i mu_i + B_j mu_j)
    z_p  = -0.5 log det(2*pi*(Lc_i+Lc_j)) - 0.5 dmu^T inv(Lc_i+Lc_j) dmu
    logprob_p(x) = c_p - ||V_p x - b_p||^2,  V_p = W_p/sqrt(2), b_p = V_p mu_p
    c_p  = z_p + logw_i + logw_j - 0.5 (D log 2pi + logdet sigma_p)
           + log 2 if i<j
  output = log( sum_{n,p} exp(logprob_p(x_n)) )   [exp in fp32, matching the
           reference's unshifted fp32 exp underflow behavior exactly]

Device (per core, data-parallel over 8 cores, 16384 samples each), per
512-sample supertile, with the bias folded into the matmul via an
augmented contraction row (x~ = [x; 1], Wcat row 32 = -b):
  PE : 4 transposes X~ -> Xt [33, 512];  9 fp32r matmuls
       Y'_c = Wcat_c^T Xt  [128 pk-rows, 512 n]  (Y' = V x - b);
       9 fp32r seg-sum matmuls accumulating q = sum_d Y'^2 into [36, 512]
  ACT: merged 2-bank squares for the later chunks, final
       exp(c_p - q) with accum_out = per-pair partial sums over 512 samples
  DVE: Xt copy + PSUM->SBUF copies for the early chunks + one square
  GPS: squares for the remaining early chunks (SBUF->SBUF)
Host sums all partials in float64 and takes log.
"""

import os
import numpy as np
from contextlib import ExitStack

import concourse.bass as bass
import concourse.tile as tile
from concourse import bacc, mybir
from concourse.bass_utils import run_bass_kernel_spmd

F32 = mybir.dt.float32
F32R = mybir.dt.float32r
AF = mybir.ActivationFunctionType

N, K, D = 131072, 8, 32
NCORES = 8
NPER = N // NCORES          # 16384 samples per core
ST = 512                    # supertile size
NST = NPER // ST            # 32 supertiles per core
PAIRS = [(i, j) for i in range(K) for j in range(i, K)]
P = len(PAIRS)              # 36
NCHUNK = (P * D) // 128     # 9 chunks of 128 pk-rows (4 pairs each)

_NCOPY = int(os.environ.get("K_NCOPY", "3"))    # chunks squared off-ACT
_NPOOL = int(os.environ.get("K_NPOOL", "2"))    # of those, on GpSimd
_PSA = int(os.environ.get("K_PSA", "1"))
_PSYP = int(os.environ.get("K_PSYP", "2"))      # 2-bank pair tiles
_PSY1 = int(os.environ.get("K_PSY1", "2"))      # single-bank tiles
_PSQ = int(os.environ.get("K_PSQ", "1"))
_XS = int(os.environ.get("K_XS", "3"))
_XT = int(os.environ.get("K_XT", "3"))
_Y2 = int(os.environ.get("K_Y2", "8"))
_GRP = int(os.environ.get("K_GRP", "2"))        # ACT square merge width


def _host_constants(mu, L, weights):
    mu = np.asarray(mu, dtype=np.float64)
    L = np.asarray(L, dtype=np.float64)
    weights = np.asarray(weights, dtype=np.float64)
    LOG2PI = float(np.log(2.0 * np.pi))

    A = np.einsum('kab,kcb->kac', L, L) + np.eye(D)
    Lc = np.linalg.cholesky(A)
    invLc = np.linalg.inv(Lc)
    wmax = weights.max()
    logw = weights - (np.log(np.sum(np.exp(weights - wmax))) + wmax)

    V = np.zeros((P, D, D))
    b = np.zeros((P, D))
    c = np.zeros(P)
    for p, (i, j) in enumerate(PAIRS):
        Bi, Bj = invLc[i], invLc[j]
        Wp = Bi + Bj
        mu_p = np.linalg.solve(Wp, Bi @ mu[i] + Bj @ mu[j])
        S = Lc[i] + Lc[j]
        dmu = mu[i] - mu[j]
        quad = -0.5 * dmu @ np.linalg.solve(S, dmu)
        z = -0.5 * (D * LOG2PI + np.sum(np.log(np.abs(np.diag(S))))) + quad
        logdet_sigma = -np.sum(np.log(np.diag(Wp)))
        Vp = Wp / np.sqrt(2.0)
        V[p] = Vp
        b[p] = Vp @ mu_p
        c[p] = (z + logw[i] + logw[j] - 0.5 * (D * LOG2PI + logdet_sigma)
                + (np.log(2.0) if i < j else 0.0))

    # Wcat[0:32, p*32+d] = V_p[d, :];  Wcat[32, p*32+d] = -b_p[d]
    Wcat = np.zeros((D + 1, P * D), dtype=np.float32)
    for p in range(P):
        Wcat[0:D, p * D:(p + 1) * D] = V[p].T.astype(np.float32)
        Wcat[D, p * D:(p + 1) * D] = (-b[p]).astype(np.float32)

    cdev = c.astype(np.float32)

    E9 = np.zeros((128, NCHUNK, P), dtype=np.float32)
    for ch in range(NCHUNK):
        for j in range(4):
            E9[j * D:(j + 1) * D, ch, 4 * ch + j] = 1.0

    ident = np.eye(128, dtype=np.float32)
    ones = np.ones((128, 4, 1), dtype=np.float32)
    return Wcat, cdev, E9, ident, ones


def _build_kernel():
    nc = bacc.Bacc("TRN2", target_bir_lowering=False, debug=False,
                   num_devices=NCORES)
    X_d = nc.dram_tensor("X", [NPER, D], F32R, kind="ExternalInput").ap()
    W_d = nc.dram_tensor("Wcat", [D + 1, P * D], F32R,
                         kind="ExternalInput").ap()
    c_d = nc.dram_tensor("cdev", [P], F32, kind="ExternalInput").ap()
    E_d = nc.dram_tensor("E9", [128, NCHUNK, P], F32R,
                         kind="ExternalInput").ap()
    I_d = nc.dram_tensor("ident", [128, 128], F32R, kind="ExternalInput").ap()
    O_d = nc.dram_tensor("ones", [128, 4, 1], F32R, kind="ExternalInput").ap()
    acc_o = nc.dram_tensor("acc", [P, NST], F32, kind="ExternalOutput").ap()

    act_chunks = list(range(_NCOPY, NCHUNK))
    pairs = []
    i = 0
    while len(act_chunks) - i >= _GRP and _GRP > 1:
        pairs.append(tuple(act_chunks[i:i + _GRP]))
        i += _GRP
    singles = act_chunks[i:]

    with tile.TileContext(nc) as tc, ExitStack() as ctx:
        const = ctx.enter_context(tc.tile_pool(name="const", bufs=1))
        xs_pool = ctx.enter_context(tc.tile_pool(name="xs", bufs=_XS))
        xt_sb_pool = ctx.enter_context(tc.tile_pool(name="xtsb", bufs=_XT))
        y2_pool = ctx.enter_context(tc.tile_pool(name="y2", bufs=_Y2))
        psA = ctx.enter_context(tc.tile_pool(name="psA", bufs=_PSA,
                                             space="PSUM"))
        psYp = ctx.enter_context(tc.tile_pool(name="psYp", bufs=_PSYP,
                                              space="PSUM"))
        psY1 = ctx.enter_context(tc.tile_pool(name="psY1", bufs=_PSY1,
                                              space="PSUM"))
        psQ = ctx.enter_context(tc.tile_pool(name="psQ", bufs=_PSQ,
                                             space="PSUM"))

        ident = const.tile([128, 128], F32R)
        nc.sync.dma_start(ident[:], I_d)
        w_sb = const.tile([D + 1, P * D], F32R)
        nc.sync.dma_start(w_sb[:], W_d)
        c_sb = const.tile([P, 1], F32)
        e_sb = const.tile([128, NCHUNK, P], F32R)

        acc_sb = const.tile([P, NST], F32)
        junk = const.tile([P, ST], F32)

        for s in range(NST):
            ncopy_s = _NCOPY
            act_chunks_s = list(range(ncopy_s, NCHUNK))
            pairs_s = []
            i = 0
            while len(act_chunks_s) - i >= _GRP and _GRP > 1:
                pairs_s.append(tuple(act_chunks_s[i:i + _GRP]))
                i += _GRP
            singles_s = act_chunks_s[i:]
            xs = xs_pool.tile([128, 4, D + 1], F32R)
            nc.sync.dma_start(
                xs[:, :, 0:D],
                X_d[s * ST:(s + 1) * ST, :].rearrange("(t p) d -> p t d",
                                                      p=128))
            nc.sync.dma_start(xs[:, :, D:D + 1], O_d)

            xt_ps = psA.tile([D + 1, ST], F32R)
            for t in range(4):
                nc.tensor.matmul(xt_ps[:, t * 128:(t + 1) * 128],
                                 xs[:, t, :], ident[:], is_transpose=True)
            xt_sb = xt_sb_pool.tile([D + 1, ST], F32R)
            nc.vector.tensor_copy(xt_sb[:], xt_ps[:])

            if s == 0:
                # defer bulky segsum/exp constants behind the first X tile
                nc.sync.dma_start(e_sb[:], E_d)
                nc.sync.dma_start(c_sb[:], c_d.rearrange("(p o) -> p o", o=1))

            q_ps = psQ.tile([P, ST], F32)
            seg = [0]         # segsum accumulation index

            def segsum(ch, y2ap, q_ps=q_ps, seg=seg):
                nc.tensor.matmul(q_ps[:], e_sb[:, ch, :], y2ap,
                                 start=(seg[0] == 0),
                                 stop=(seg[0] == NCHUNK - 1))
                seg[0] += 1

            # off-ACT chunks: produce first (longest latency chains), but
            # accumulate them last so they don't gate the segsum order
            deferred = []
            for k in range(ncopy_s):
                ch = k
                y_ps = psY1.tile([128, ST], F32)
                nc.tensor.matmul(y_ps[:], w_sb[:, ch * 128:(ch + 1) * 128],
                                 xt_sb[:], start=True, stop=True)
                y_cp = y2_pool.tile([128, ST], F32, tag="ycp")
                nc.vector.tensor_copy(y_cp[:], y_ps[:])
                y2 = y2_pool.tile([128, ST], F32R, tag="y2s")
                if k < ncopy_s - _NPOOL:
                    nc.vector.tensor_mul(y2[:], y_cp[:], y_cp[:])
                else:
                    nc.gpsimd.tensor_mul(y2[:], y_cp[:], y_cp[:])
                deferred.append((ch, y2))

            # ACT groups (merged multi-bank squares)
            for grp in pairs_s:
                yp_ps = psYp.tile([128, _GRP, ST], F32)
                for gi, cg in enumerate(grp):
                    nc.tensor.matmul(yp_ps[:, gi, :],
                                     w_sb[:, cg * 128:(cg + 1) * 128],
                                     xt_sb[:], start=True, stop=True)
                y2p = y2_pool.tile([128, _GRP, ST], F32R, tag="y2p")
                nc.scalar.activation(y2p[:], yp_ps[:], AF.Square)
                for gi, cg in enumerate(grp):
                    segsum(cg, y2p[:, gi, :])

            for ch in singles_s:
                y_ps = psY1.tile([128, ST], F32)
                nc.tensor.matmul(y_ps[:], w_sb[:, ch * 128:(ch + 1) * 128],
                                 xt_sb[:], start=True, stop=True)
                y2 = y2_pool.tile([128, ST], F32R, tag="y2s")
                nc.scalar.activation(y2[:], y_ps[:], AF.Square)
                segsum(ch, y2[:])

            for (ch, y2) in deferred:
                segsum(ch, y2[:])

            nc.scalar.activation(junk[:], q_ps[:], AF.Exp,
                                 bias=c_sb[:], scale=-1.0,
                                 accum_out=acc_sb[:, s:s + 1])

        nc.sync.dma_start(acc_o, acc_sb[:])

    nc.compile()
    return nc


_CACHED = {}


def kernel(X, mu, L, weights, it=None, **_unused):
    X = np.ascontiguousarray(np.asarray(X, dtype=np.float32))
    Wcat, cdev, E9, ident, ones = _host_constants(mu, L, weights)

    if "nc" not in _CACHED:
        _CACHED["nc"] = _build_kernel()
    nc = _CACHED["nc"]

    in_maps = []
    for cid in range(NCORES):
        in_maps.append({
            "X": X[cid * NPER:(cid + 1) * NPER],
            "Wcat": Wcat, "cdev": cdev, "E9": E9, "ident": ident,
            "ones": ones,
        })
    res = run_bass_kernel_spmd(nc, in_maps, core_ids=list(range(NCORES)))

    total = np.float64(0.0)
    for r in res.results:
        total += np.sum(r["acc"].astype(np.float64))
    with np.errstate(divide="ignore"):
        out = np.float32(np.log(total))
    return np.asarray(out, dtype=np.float32)


if __name__ == "__main__":
    rng = np.random.default_rng(0)
    ins = {
        "X": rng.standard_normal((N, D)).astype(np.float32),
        "mu": rng.standard_normal((K, D)).astype(np.float32),
        "L": rng.standard_normal((K, D, D)).astype(np.float32),
        "weights": rng.standard_normal((K,)).astype(np.float32),
        "it": 0,
    }
    print("kernel out:", kernel(**ins))
